# revision 46
# baseline (speedup 1.0000x reference)
"""BotRGCN on 8 trn2 NeuronCores (SPMD, raw Bacc).

Nodes row-sharded 8 ways (12500/core, padded to 12800). Host pre-packs a
feature-major bf16 matrix dtb=[des|tweet|num|cat|ones]^T [1664, SP] per core,
so Phase A is 13 accumulating bf16 matmuls per 512-node chunk (no device-side
transposes of the wide inputs). RGCN layers: AllGather bf16 node features ->
shared compact table [102400, 32]; per-(relation, src-row%4-bucket)
degree-sorted ELL gathers via the gpsimd dma_gather library op (int16 idx =
row//4 against a [25600,128]-strided view, 64B elems, thousands of idx per
instruction); DVE accumulates buckets in bucket-sorted order; partial sums
roundtrip through padded DRAM scratch and come back via dma_gather unpermute
in canonical order; DVE combines 4 buckets + scales by 1/cnt; PE re-transposes
to feature-major for the dense tail matmuls.
"""
import sys
sys.path.insert(0, "/opt/trn_rl_repo")
from contextlib import ExitStack

import numpy as np
import ml_dtypes

from concourse import bacc, bass, mybir
from concourse import ap_utils
from concourse import library_config
from concourse.bass_utils import run_bass_kernel_spmd

F32 = mybir.dt.float32
BF16 = mybir.dt.bfloat16
F16 = mybir.dt.float16
I16 = mybir.dt.int16
ACOPY = mybir.ActivationFunctionType.Copy

N_CORES = 8
NUM_REL = 2
NBKT = 4           # src-row % 4 buckets (int16 idx = row//4, 256B stride view)
CBUF = 60          # gather piece columns (7680 idx; 2 fit the desc ring)
NCH = 13           # contraction chunks in phase A (1664 = 13*128)
DTR = NCH * 128
DUMMY_TQ = 14325  # table row 57300+b (core-0 pad rows, zeroed)
_QROT = True
_QROT_UNP = True
_DEBUG_AGG = False    # (12500+b)//4 for b<4: zeroed pad row of shard 0, any bucket


def dma_gather64(eng, out_ap, in_ap, idxs_ap, num_idxs, elem_size, elem_step,
                 queue_num=0):
    """bass dma_gather minus the 256B elem assert (non-transpose, HBM src)."""
    assert idxs_ap.dtype == mybir.dt.int16
    assert in_ap.dtype == out_ap.dtype
    assert ap_utils.ap_is_contiguous(in_ap.ap[1:])
    assert ap_utils.ap_is_contiguous(out_ap.ap[1:])
    assert ap_utils.ap_is_contiguous(idxs_ap.ap[1:])
    assert in_ap.ap[-1][1] == out_ap.ap[-1][1] == elem_size
    assert out_ap.ap[0][1] * out_ap.ap[1][1] == ((num_idxs + 127) // 128) * 128
    assert in_ap.ap[0][0] == elem_step
    stride_bytes = elem_step * mybir.dt.size(in_ap.dtype)
    sb256 = stride_bytes // 256
    assert sb256 * 256 == stride_bytes and sb256 < 256
    _in_ap = eng.lower_ap_dma(in_ap, for_custom_bir_dma=True)
    _idxs_ap = eng.lower_ap(idxs_ap)
    _out_ap = eng.lower_ap(out_ap)
    return eng.add_instruction(
        mybir.InstDMAGatherAnt(
            name=eng.bass.get_next_instruction_name(),
            ins=[*_in_ap, _idxs_ap, eng.lower_val_access(eng.to_reg(num_idxs))],
            outs=[_out_ap],
            transpose=False,
            num_idxs=num_idxs,
            elem_size=elem_size,
            stride_bytes_256=sb256,
            gen_mode=0,
            single_packet=False,
            queue_num=queue_num,
            sbuf_tokens_per_rank=0,
            sbuf_free_dim_per_rank=0,
            sbuf_free_dim_pad_per_rank=0,
            sbuf_byte_offset=0,
        ))


class Cfg:
    def __init__(self, shard_real, shard_pad, pieces, copy_cols, ixbase, ubase, t16):
        self.shard_real = shard_real
        self.shard_pad = shard_pad
        self.n_super = shard_pad // 512
        self.nt = shard_pad // 128
        self.tabv = N_CORES * shard_pad
        # pieces[r][b] = [(ck, adds)]; adds = [(agg_blk0, msg_blk0, nblk)]
        self.pieces = pieces
        self.copy_cols = copy_cols   # [r][b]: plane-0 col count (copied, not added)
        self.ixbase = ixbase         # [r][b][pi] -> idx16 col base
        self.ubase = ubase           # [r][b] -> idx16 col base of unperm block
        self.t16 = t16


def build_bass(cfg: Cfg):
    nc = bacc.Bacc("TRN2", debug=False, num_swdge_queues=4)
    mmctx = ExitStack()
    SP = cfg.shard_pad
    NT = cfg.nt
    NS = cfg.n_super
    TABV = cfg.tabv
    UCOLS = SP // 16

    dtb_in = nc.declare_dram_parameter("dtb", [DTR, SP], BF16, isOutput=False)
    wall_in = nc.declare_dram_parameter("wall", [DTR, 32], BF16, isOutput=False)
    idx16_in = nc.declare_dram_parameter("idx16", [128, cfg.t16], I16, isOutput=False)
    coef_in = nc.declare_dram_parameter("coefnm", [128, NUM_REL, NT], F16, isOutput=False)
    wsm_in = nc.declare_dram_parameter("wsm", [6, 33, 32], BF16, isOutput=False)
    id128_in = nc.declare_dram_parameter("id128", [128, 128], F16, isOutput=False)
    id32_in = nc.declare_dram_parameter("id32", [32, 32], BF16, isOutput=False)
    out_ext = nc.declare_dram_parameter("out", [2, SP], F32, isOutput=True)
    if _DEBUG_AGG:
        dbg_bkt = [[nc.declare_dram_parameter(f"dbgb{r}_{b}", [128, NT * 32], F16, isOutput=True)
                    for b in range(NBKT)] for r in range(NUM_REL)]
        dbg_ext = [nc.declare_dram_parameter(f"dbg{r}", [128, NT, 32], F16, isOutput=True)
                   for r in range(NUM_REL)]
        dbg_land = [nc.declare_dram_parameter(f"dbgl{b}", [128, NT, 32], F16, isOutput=True)
                    for b in range(NBKT)]

    shard_ag = nc.dram_tensor("shard_ag", [SP, 32], F16)
    table = nc.dram_tensor("table", [TABV, 32], F16, addr_space="Shared")
    scratch = [[nc.dram_tensor(f"scratch{r}_{b}", [SP, 128], F16)
                for b in range(NBKT)] for r in range(NUM_REL)]

    live = []

    def sb(name, shape, dt):
        cm = nc.sbuf_tensor(name, shape, dt)
        t = cm.__enter__()
        live.append(cm)
        return t

    def psum(name, shape):
        cm = nc.psum_tensor(name, shape, F32)
        t = cm.__enter__()
        live.append(cm)
        return t

    sb_wall = sb("sb_wall", [128, NCH, 32], BF16)
    xT = sb("xT", [33, SP], BF16)
    aggT0 = sb("aggT0", [32, SP], BF16)
    aggT1 = sb("aggT1", [32, SP], BF16)
    agg2 = [sb(f"agg{i}", [128, NT * 32], F16) for i in range(4)]
    agg_k = sb("agg_k", [128, NT * 32], F16)
    agg_c = [sb(f"agg_c{r}", [128, NT, 32], F16) for r in range(NUM_REL)]
    land = [sb(f"land{i}", [128, NT, 32], F16) for i in range(4)]
    sb_shard = sb("sb_shard", [128, NT, 32], F16)
    sb_iu = [sb(f"sb_iu{i}", [128, UCOLS], I16) for i in range(NUM_REL * NBKT)]
    sb_coef = sb("sb_coef", [128, NUM_REL, NT], F16)
    sb_wsm = sb("sb_wsm", [33, 6, 32], BF16)
    sb_id128 = sb("sb_id128", [128, 128], F16)
    sb_id32 = sb("sb_id32", [32, 32], BF16)
    sb_x3T = sb("sb_x3T", [33, 512], BF16)
    sb_lg = sb("sb_lg", [2, 2, 512], F32)

    # sb_dtb is phase-A-only: allocate it, free it, and let the gather-phase
    # buffers (msgs/sb_ix) overlay its SBUF range. All writers of the overlay
    # are ordered after phase A's last sb_dtb read (sync waits s_mm>=2*NS
    # before the first idx stage; gathers wait the AllGather of x).
    dtb_cm = nc.sbuf_tensor("sb_dtb", [128, 2, NCH, 512], BF16)
    sb_dtb = dtb_cm.__enter__()
    dtb_cm.__exit__(None, None, None)
    NBUF = 8
    msgs2 = [sb(f"msgs{i}", [128, CBUF, 32], F16) for i in range(NBUF)]
    sb_ix = [sb(f"sb_ix{i}", [128, CBUF * 8], I16) for i in range(NBUF)]

    pb = [psum(f"pb{i}", [128, 512]) for i in range(8)]
    pbx = pb[5][:, :].bitcast(BF16)

    # compact table viewed as [TABV/4, 128]; bucket b = cols [b*32, b*32+32)
    tab_q = table.ap().rearrange("(q f) d -> q (f d)", f=4)
    scr_v = [[scratch[r][b].ap()[:, 0:32] for b in range(NBKT)] for r in range(NUM_REL)]

    plan = {"sync": [], "pe": [], "act": [], "dve": [], "gp": []}

    def op(engine, fn):
        plan[engine].append(fn)

    class Sem:
        def __init__(self, name):
            cm = nc.semaphore(name)
            self.h = cm.__enter__()
            live.append(cm)
            self.n = 0

        def inc(self, inst, k=1):
            inst.then_inc(self.h, k)

        def pinc(self, k=1):
            self.n += k
            return self.n

    s_load = Sem("s_load")
    s_ld = [Sem("s_ld0"), Sem("s_ld1")]
    s_lr = Sem("s_lr")
    s_gq = [Sem(f"s_gq{i}") for i in range(8)]
    s_gu = Sem("s_gu")
    s_uq = [Sem(f"s_uq{i}") for i in range(8)]
    s_ix = Sem("s_ix")
    s_sc = Sem("s_sc")
    s_tp = Sem("s_tp")
    s_cp = Sem("s_cp")
    s_mm = Sem("s_mm")
    s_x1 = Sem("s_x1")
    s_gp = Sem("s_gp")
    s_cc = Sem("s_cc")
    s_dve = Sem("s_dve")
    s_sh = Sem("s_sh")

    def W(engine, sem, val):
        if val > 0:
            op(engine, lambda eng, s=sem, v=val: eng.wait_ge(s.h, v))

    # ---------------- constants ----------------
    def c_loads(eng):
        for rr in range(NUM_REL):
            for bb in range(NBKT):
                ub = cfg.ubase[rr][bb]
                eng.dma_start(out=sb_iu[rr * NBKT + bb][:, :],
                              in_=idx16_in[:, ub:ub + UCOLS]).then_inc(s_load.h, 16)
        eng.dma_start(out=sb_coef[:], in_=coef_in[:, :, :]).then_inc(s_load.h, 16)
        eng.dma_start(out=sb_wall[:], in_=wall_in.ap().rearrange("(c p) m -> p c m", p=128)).then_inc(s_load.h, 16)
        eng.dma_start(out=sb_wsm[:], in_=wsm_in.ap().rearrange("c p m -> p c m")).then_inc(s_load.h, 16)
        eng.dma_start(out=sb_id128[:], in_=id128_in[:, :]).then_inc(s_load.h, 16)
        eng.dma_start(out=sb_id32[:], in_=id32_in[:, :]).then_inc(s_load.h, 16)
    op("sync", c_loads)
    s_load.pinc((5 + NUM_REL * NBKT) * 16)
    NCONST = s_load.n

    op("gp", lambda eng: eng.load_library(library_config.mlp))

    def init_ones(eng):
        eng.memset(xT[32:33, :], 1.0)
        s_dve.inc(eng.memset(sb_x3T[32:33, :], 1.0))
    op("dve", init_ones)
    s_dve.pinc()
    NINIT = s_dve.n

    # =======================================================
    # Phase A
    # =======================================================
    for i in range(NS):
        buf = i % 2
        sl = slice(i * 512, (i + 1) * 512)

        if i >= 2:
            W("sync", s_mm, 2 * i - 3)

        def ld(eng, i=i, buf=buf):
            eng.dma_start(
                out=sb_dtb[:, buf, :, :],
                in_=dtb_in.ap()[:, i * 512:(i + 1) * 512].rearrange(
                    "(c p) n -> p c n", p=128)).then_inc(s_ld[buf].h, 16)
        op("sync", ld)
        s_ld[buf].pinc(16)

        if i == 0:
            W("pe", s_load, NCONST)
        W("pe", s_ld[buf], 16 * (i // 2 + 1))
        if i >= 1:
            W("pe", s_lr, 2 * i - 1)

        def pe_mm(eng, buf=buf):
            last = None
            for c in range(NCH):
                last = nc.tensor.matmul(pb[6][0:32, :], sb_wall[:, c, :],
                                        sb_dtb[:, buf, c, :],
                                        start=(c == 0), stop=(c == NCH - 1))
            s_mm.inc(last)
        op("pe", pe_mm)
        s_mm.pinc()

        W("pe", s_x1, 2 * i + 1)
        if i >= 1:
            W("pe", s_lr, 2 * i)
        if i == 0:
            W("pe", s_dve, NINIT)

        def pe_wi(eng, sl=sl):
            last = nc.tensor.matmul(pb[7][0:32, :], sb_wsm[:, 0, :],
                                    xT[0:33, sl], start=True, stop=True)
            s_mm.inc(last)
        op("pe", pe_wi)
        s_mm.pinc()

        W("pe", s_x1, 2 * i + 2)
        if i >= 1:
            W("pe", s_sh, i)
        if i == NS - 1 and cfg.shard_real < SP:
            W("pe", s_dve, NINIT + 1)

        def pe_x1t(eng, i=i):
            last = None
            for t in range(4):
                last = nc.tensor.transpose(
                    out=pbx[:, t * 32:(t + 1) * 32],
                    in_=xT[0:32, i * 512 + t * 128:i * 512 + (t + 1) * 128],
                    identity=sb_id32[:])
            s_tp.inc(last)
        op("pe", pe_x1t)
        s_tp.pinc()

        # ---- ACT / DVE ----
        W("act", s_mm, 2 * i + 1)
        op("act", lambda eng, sl=sl: s_lr.inc(eng.activation(
            out=xT[0:32, sl], in_=pb[6][0:32, :], func=ACOPY)))
        s_lr.pinc()
        W("dve", s_lr, s_lr.n)

        def act_x(eng, sl=sl):
            s_x1.inc(nc.vector.scalar_tensor_tensor(
                out=xT[0:32, sl], in0=xT[0:32, sl], scalar=0.01, in1=xT[0:32, sl],
                op0=mybir.AluOpType.mult, op1=mybir.AluOpType.max))
        op("dve", act_x)
        s_x1.pinc()

        W("act", s_mm, 2 * i + 2)
        op("act", lambda eng, sl=sl: s_lr.inc(eng.activation(
            out=xT[0:32, sl], in_=pb[7][0:32, :], func=ACOPY)))
        s_lr.pinc()
        W("dve", s_lr, s_lr.n)

        def act_x1(eng, sl=sl):
            s_x1.inc(nc.vector.scalar_tensor_tensor(
                out=xT[0:32, sl], in0=xT[0:32, sl], scalar=0.01, in1=xT[0:32, sl],
                op0=mybir.AluOpType.mult, op1=mybir.AluOpType.max))
        op("dve", act_x1)
        s_x1.pinc()
        if i == NS - 1 and cfg.shard_real < SP:
            W("dve", s_x1, 2 * NS)
            op("dve", lambda eng: s_dve.inc(eng.memset(xT[0:32, cfg.shard_real:SP], 0)))
            s_dve.pinc()

        W("act", s_tp, i + 1)

        def act_sh(eng, i=i):
            s_sh.inc(eng.activation(
                out=sb_shard[:, 4 * i:4 * i + 4, :].rearrange("p a b -> p (a b)"),
                in_=pbx[:, 0:128], func=ACOPY))
        op("act", act_sh)
        s_sh.pinc()

    # =======================================================
    # RGCN layers
    # =======================================================
    # Per-queue gather machinery. HW constraint (observed): the buffer+sem set
    # of a gather stream must stay on ONE SWDGE queue; cross-queue reuse of a
    # slot sem/buffer corrupts data. So: queue q owns msgs2/sb_ix/s_gq slots
    # {2q, 2q+1} (2-deep pipeline per queue); pieces are assigned to queues by
    # greedy load balance.
    NQ = 4 if _QROT else 1
    qload = [0] * NQ
    mseq = [0] * NQ        # msg gathers emitted per queue
    stg_barrier = [False]  # first layer-1 stg waits for phase A (sb_dtb overlay)
    lastg = {}             # (q, slot) -> s_gq count of previous gather there
    lasta = {}             # (q, slot) -> s_dve count of previous acc there
    bctr = [0]             # global bucket counter
    scwN = {}            # bucket -> s_sc count after its scratch write
    combN = {}           # bucket -> s_dve count after its combine
    cpyN = {}            # bucket -> s_dve count after its agg_sb copy
    scuN = {}            # bucket -> s_sc count after its scw+stu
    upN = {}             # bucket -> s_gu count after its unperm
    atdone = {0: 0, 1: 0}   # relation -> s_tp count after its pe_at loop
    cpb = {0: 0, 1: 0}      # relation -> s_cp count before its copy chain

    def emit_bucket(r, b, layer_first):
        bk = bctr[0]
        bctr[0] += 1
        ag = agg2[bk % 4]
        # agg2[bk%4] WAR: scratch write of bucket bk-4 must have drained
        if bk >= 4:
            W("dve", s_sc, scwN[bk - 4])
        # DVE: zero the accumulator (memset is fast; DVE tensor_copy is ~10x
        # slower per element, so plane-0 "copy" segments are not worth it).
        # drain: a back-to-back RMW right after a DVE write op can read stale
        # data (loses the writer's leading bytes) -- drain between them.
        def clr(eng, ag=ag):
            eng.memset(ag[:, :], 0)
            s_dve.inc(eng.drain())
        op("dve", clr)
        s_dve.pinc()

        npieces = len(cfg.pieces[r][b])
        for pi, (ck, adds) in enumerate(cfg.pieces[r][b]):
            qn = min(range(NQ), key=lambda q: qload[q])
            qload[qn] += ck * 128
            slot = mseq[qn] % 2
            mseq[qn] += 1
            mb2 = qn * 2 + slot
            sq = mb2
            # SYNC: stage idx piece (buffer WAR: previous gather on this slot)
            if not stg_barrier[0]:
                # sb_ix/msgs overlay sb_dtb: wait for phase A's matmuls
                W("sync", s_mm, 2 * cfg.n_super)
                stg_barrier[0] = True
            if (qn, slot) in lastg:
                W("sync", s_gq[sq], lastg[(qn, slot)])
            base = cfg.ixbase[r][b][pi]

            def stg(eng, mb2=mb2, ck=ck, base=base):
                eng.dma_start(out=sb_ix[mb2][:, 0:ck * 8],
                              in_=idx16_in[:, base:base + ck * 8]).then_inc(s_ix.h, 16)
            op("sync", stg)
            s_ix.pinc(16)

            # GP: gather
            W("gp", s_ix, s_ix.n)
            if layer_first and pi == 0 and b == 0 and r == 0:
                W("gp", s_cc, s_cc.n)
            if (qn, slot) in lasta:
                W("gp", s_dve, lasta[(qn, slot)])

            def gth(eng, mb2=mb2, ck=ck, b=b, sq=sq, qn=qn):
                s_gq[sq].inc(dma_gather64(
                    eng, msgs2[mb2][:, 0:ck, :],
                    tab_q[:, b * 32:(b + 1) * 32],
                    sb_ix[mb2][:, 0:ck * 8], ck * 128, 32, 128,
                    queue_num=qn), 16)
            op("gp", gth)
            sqq = sq
            s_gq[sqq].pinc(16)
            lastg[(qn, slot)] = s_gq[sqq].n

            # DVE: accumulate
            W("dve", s_gq[sqq], s_gq[sqq].n)

            def acc(eng, adds=adds, mb2=mb2, ag=ag):
                last = None
                need_drain = False
                for j, (ab, mb, nb, isk0) in enumerate(adds):
                    if j > 0 and need_drain:
                        eng.drain()
                    last = nc.vector.tensor_tensor(
                        out=ag[:, ab * 32:(ab + nb) * 32].rearrange(
                            "p (c d) -> p c d", d=32),
                        in0=ag[:, ab * 32:(ab + nb) * 32].rearrange(
                            "p (c d) -> p c d", d=32),
                        in1=msgs2[mb2][:, mb:mb + nb, :],
                        op=mybir.AluOpType.add)
                    need_drain = nb < 8
                if need_drain:
                    last = eng.drain()
                s_dve.inc(last)
            op("dve", acc)
            s_dve.pinc()
            lasta[(qn, slot)] = s_dve.n

        # ACT: scratch write straight from agg (fp16, strided 256B rows)
        W("act", s_dve, s_dve.n)

        def scw(eng, r=r, b=b, ag=ag):
            eng.dma_start(
                out=scr_v[r][b].rearrange("(t p) d -> p t d", p=128),
                in_=ag[:, :].rearrange("p (t d) -> p t d", d=32)
            ).then_inc(s_sc.h, 16)
        op("act", scw)
        s_sc.pinc(16)
        if _DEBUG_AGG and bk >= NUM_REL * NBKT:
            def dbgw(eng, r=r, b=b, ag=ag):
                eng.dma_start(out=dbg_bkt[r][b][:, :], in_=ag[:, :]).then_inc(s_sc.h, 16)
            op("act", dbgw)
            s_sc.pinc(16)
        scwN[bk] = s_sc.n

        scuN[bk] = s_sc.n
        return bk

    def emit_unp(r, b, bk):
        # GP: unperm gather into land[bk%2] (deferred past next bucket's gathers)
        # Queue-affine: land[j] half h is always written from queue 2j+h, and
        # sem s_uq[2j+h] is only inc'd from that queue.
        W("gp", s_sc, scuN[bk])
        if bk >= 4:
            W("gp", s_dve, combN[bk - 4])
        UC = 6400

        ups = []
        for h in range(2):
            ui = 2 * (bk % 4) + h
            qn = (ui % 4) if (_QROT and _QROT_UNP) else 0
            qload[qn % NQ] += UC
            ups.append((h, ui, qn))

        def unp(eng, r=r, b=b, bk=bk, ups=tuple(ups)):
            for h, ui, qn in ups:
                c0 = h * UC
                s_uq[ui].inc(dma_gather64(
                    eng, land[bk % 4][:, c0 // 128:(c0 + UC) // 128, :], scr_v[r][b],
                    sb_iu[(r * NBKT + b)][:, c0 // 16:(c0 + UC) // 16], UC, 32, 128,
                    queue_num=qn), 16)
        op("gp", unp)
        upN[bk] = []
        for h, ui, qn in ups:
            s_uq[ui].pinc(16)
            upN[bk].append((s_uq[ui], s_uq[ui].n))

    def emit_combine(r, bk, b):
        for sem, cnt in upN[bk]:
            W("dve", sem, cnt)

        def comb(eng, bk=bk, b=b):
            if b == 0:
                eng.memset(agg_k[:, :], 0)
                eng.drain()
            s_dve.inc(nc.vector.tensor_tensor(
                out=agg_k[:, :].rearrange("p (t d) -> p t d", d=32),
                in0=agg_k[:, :].rearrange("p (t d) -> p t d", d=32),
                in1=land[bk % 4][:, :, :],
                op=mybir.AluOpType.add))
        op("dve", comb)
        s_dve.pinc()
        combN[bk] = s_dve.n
        if _DEBUG_AGG and bk < NBKT:
            W("act", s_dve, combN[bk])

            def dbgl(eng, bk=bk):
                eng.dma_start(out=dbg_land[bk][:, :, :],
                              in_=land[bk % 4][:, :, :]).then_inc(s_sc.h, 16)
            op("act", dbgl)
            s_sc.pinc(16)

    def emit_layer(layer):
        # AllGather in two halves so the first overlaps the producer chain
        HN = SP // 2
        HT = NT // 2
        # table is HALF-MAJOR: rows [h*8*HN + c*HN + v] (host idx math matches)
        half_thr = s_sh.n - 12  # shard copies for nodes [0, HN) done
        for h in range(2):
            W("gp", s_sh, half_thr if h == 0 else s_sh.n)

            def shard_h(eng, h=h):
                s_gp.inc(eng.dma_start(
                    out=shard_ag.ap()[h * HN:(h + 1) * HN, :].rearrange(
                        "(t p) d -> p t d", p=128),
                    in_=sb_shard[:, h * HT:(h + 1) * HT, :]), 16)
            op("gp", shard_h)
            s_gp.pinc(16)
            W("gp", s_gp, s_gp.n)

            def cc_h(eng, h=h):
                s_cc.inc(eng.collective_compute(
                    "AllGather", mybir.AluOpType.bypass,
                    ins=[shard_ag.ap()[h * HN:(h + 1) * HN, :]],
                    outs=[table.ap()[h * N_CORES * HN:(h + 1) * N_CORES * HN, :]],
                    replica_groups=[list(range(N_CORES))]))
            op("gp", cc_h)
            s_cc.pinc()
        gp_shard_done = s_gp.n
        W("gp", s_cc, s_cc.n)

        def emit_rel_tail(r):
            # DVE: scale canonical agg_k by 1/cnt -> agg_c[r] fp16 (one op)
            W("dve", s_tp, atdone[r])

            def scl(eng, r=r):
                cb = sb_coef[:, r:r + 1, :].rearrange("p a t -> p t a").to_broadcast(
                    [128, NT, 32])
                s_dve.inc(nc.vector.tensor_tensor(
                    out=agg_c[r][:, :, :],
                    in0=agg_k[:, :].rearrange("p (t d) -> p t d", d=32),
                    in1=cb, op=mybir.AluOpType.mult))
            op("dve", scl)
            s_dve.pinc()
            sclN = s_dve.n
            if _DEBUG_AGG and r in (0, 1) and atdone[r] == 0:
                W("act", s_dve, sclN)

                def dbgs(eng, r=r):
                    eng.dma_start(out=dbg_ext[r][:, :, :],
                                  in_=agg_c[r][:, :, :]).then_inc(s_sc.h, 16)
                op("act", dbgs)
                s_sc.pinc(16)

            # PE: transpose agg_c[r] (fp16) -> aggT (bf16 ^T) via ACT copies
            aggT = aggT0 if r == 0 else aggT1
            W("pe", s_dve, sclN)
            C0 = s_cp.n
            cpb[r] = C0
            T0 = s_tp.n
            for g in range(NT // 4):
                bankx = pb[1 + (g % 2)][:, :].bitcast(F16)
                W("pe", s_cp, C0 + g - 1 if g >= 2 else C0)

                def pe_at(eng, g=g, bankx=bankx, r=r):
                    last = None
                    for t in range(4):
                        n = g * 4 + t
                        last = nc.tensor.transpose(out=bankx[0:32, t * 128:(t + 1) * 128],
                                                   in_=agg_c[r][:, n, :],
                                                   identity=sb_id128[:])
                    s_tp.inc(last)
                op("pe", pe_at)
                s_tp.pinc()
                W("act", s_tp, T0 + g + 1)

                def act_at(eng, g=g, bankx=bankx, aggT=aggT):
                    s_cp.inc(eng.activation(out=aggT[:, g * 512:(g + 1) * 512],
                                            in_=bankx[0:32, 0:512], func=ACOPY))
                op("act", act_at)
                s_cp.pinc()
            atdone[r] = s_tp.n

        # Buckets with unp/combine deferred 2 buckets back (keeps 4 queues fed:
        # the unp's SEQ waits would otherwise stall the next buckets' gathers)
        unp_pend = []
        for r in range(NUM_REL):
            for b in range(NBKT):
                bk = emit_bucket(r, b, layer == 1 or layer == 2)
                unp_pend.append((r, b, bk))
                if len(unp_pend) > 1:
                    rr, bb, kk = unp_pend.pop(0)
                    emit_unp(rr, bb, kk)
                    emit_combine(rr, kk, bb)
                    if bb == NBKT - 1:
                        emit_rel_tail(rr)
        while unp_pend:
            rr, bb, kk = unp_pend.pop(0)
            emit_unp(rr, bb, kk)
            emit_combine(rr, kk, bb)
            if bb == NBKT - 1:
                emit_rel_tail(rr)

        # dense tail (chunk ch only needs aggT cols [512ch,512ch+512): wait the
        # transpose-copy chains per chunk instead of in full)
        W("pe", s_x1, s_x1.n)
        X0 = s_x1.n
        for ch in range(NS):
            bank = pb[3 + (ch % 2)]
            W("pe", s_cp, cpb[1] + ch + 1)
            if ch >= 2:
                W("pe", s_x1, X0 + ch - 1)

            def pe_tail(eng, ch=ch, bank=bank):
                sl = slice(ch * 512, (ch + 1) * 512)
                nc.tensor.matmul(bank[0:32, :], sb_wsm[:, 1, :], xT[0:33, sl],
                                 start=True, stop=False)
                nc.tensor.matmul(bank[0:32, :], sb_wsm[0:32, 2, :], aggT0[:, sl],
                                 start=False, stop=False)
                last = nc.tensor.matmul(bank[0:32, :], sb_wsm[0:32, 3, :], aggT1[:, sl],
                                        start=False, stop=True)
                s_mm.inc(last)
            op("pe", pe_tail)
            s_mm.pinc()
            W("act", s_mm, s_mm.n)

            def act_tail(eng, ch=ch, bank=bank):
                s_x1.inc(eng.activation(out=xT[0:32, ch * 512:(ch + 1) * 512],
                                        in_=bank[0:32, :], func=ACOPY))
            op("act", act_tail)
            s_x1.pinc()

        if layer == 1:
            S0 = s_sh.n
            X1 = X0
            for ch in range(NS):
                W("pe", s_x1, X1 + ch + 1)
                if ch >= 1:
                    W("pe", s_sh, S0 + ch)
                if ch == NS - 1 and cfg.shard_real < SP:
                    W("dve", s_x1, X1 + NS)
                    op("dve", lambda eng: s_dve.inc(eng.memset(xT[0:32, cfg.shard_real:SP], 0)))
                    s_dve.pinc()
                    W("pe", s_dve, s_dve.n)

                def pe_sh(eng, ch=ch):
                    last = None
                    for t in range(4):
                        last = nc.tensor.transpose(
                            out=pbx[:, t * 32:(t + 1) * 32],
                            in_=xT[0:32, ch * 512 + t * 128:ch * 512 + (t + 1) * 128],
                            identity=sb_id32[:])
                    s_tp.inc(last)
                op("pe", pe_sh)
                s_tp.pinc()
                W("act", s_tp, s_tp.n)
                if ch == 0:
                    W("act", s_gp, gp_shard_done)

                def act_sh2(eng, ch=ch):
                    s_sh.inc(eng.activation(
                        out=sb_shard[:, 4 * ch:4 * ch + 4, :].rearrange("p a b -> p (a b)"),
                        in_=pbx[:, 0:128], func=ACOPY))
                op("act", act_sh2)
                s_sh.pinc()

    emit_layer(1)
    emit_layer(2)

    # =======================================================
    # head
    # =======================================================
    W("pe", s_x1, s_x1.n)
    XH = s_x1.n
    GH = s_gp.n
    for ch in range(NS):
        bank = pb[3 + (ch % 2)]
        if ch >= 1:
            W("pe", s_x1, XH + 2 * ch)

        def pe_h1(eng, ch=ch, bank=bank):
            s_mm.inc(nc.tensor.matmul(bank[0:32, :], sb_wsm[:, 4, :],
                                      xT[0:33, ch * 512:(ch + 1) * 512], start=True, stop=True))
        op("pe", pe_h1)
        s_mm.pinc()
        W("act", s_mm, s_mm.n)
        op("act", lambda eng, bank=bank: s_lr.inc(eng.activation(
            out=sb_x3T[0:32, :], in_=bank[0:32, :], func=ACOPY)))
        s_lr.pinc()
        W("dve", s_lr, s_lr.n)

        def act_h1(eng):
            s_x1.inc(nc.vector.scalar_tensor_tensor(
                out=sb_x3T[0:32, :], in0=sb_x3T[0:32, :], scalar=0.01,
                in1=sb_x3T[0:32, :], op0=mybir.AluOpType.mult, op1=mybir.AluOpType.max))
        op("dve", act_h1)
        s_x1.pinc()
        W("pe", s_x1, s_x1.n)

        def pe_h2(eng, bank=bank):
            s_mm.inc(nc.tensor.matmul(bank[0:2, :], sb_wsm[:, 5, 0:2],
                                      sb_x3T[0:33, :], start=True, stop=True))
        op("pe", pe_h2)
        s_mm.pinc()
        W("act", s_mm, s_mm.n)
        if ch >= 2:
            W("act", s_gp, GH + (ch - 1) * 16)

        def act_h2(eng, ch=ch, bank=bank):
            s_x1.inc(eng.activation(out=sb_lg[:, ch % 2, :], in_=bank[0:2, :],
                                    func=ACOPY))
        op("act", act_h2)
        s_x1.pinc()
        W("gp", s_x1, s_x1.n)

        def gp_out(eng, ch=ch):
            s_gp.inc(eng.dma_start(out=out_ext[:, ch * 512:(ch + 1) * 512],
                                   in_=sb_lg[:, ch % 2, :]), 16)
        op("gp", gp_out)
        s_gp.pinc(16)
    W("gp", s_gp, s_gp.n)

    with nc.Block() as block:
        @block.sync
        def _(eng):
            for f in plan["sync"]:
                f(eng)

        @block.tensor
        def _(eng):
            for f in plan["pe"]:
                f(eng)

        @block.scalar
        def _(eng):
            for f in plan["act"]:
                f(eng)

        @block.vector
        def _(eng):
            for f in plan["dve"]:
                f(eng)

        @block.gpsimd
        def _(eng):
            for f in plan["gp"]:
                f(eng)

    nc.compile()
    nc._live_refs = (live, mmctx)
    return nc


# =======================================================
# Host side
# =======================================================
def _build_structures(edge_index, edge_type, shard_real=12500, shard_pad=12800):
    SP = shard_pad
    src = edge_index[0].astype(np.int64)
    dst = edge_index[1].astype(np.int64)
    et = edge_type.astype(np.int64)
    owner = dst // shard_real
    ldst = dst % shard_real
    # half-major table: row = half*(8*SP/2) + core*(SP/2) + (v - half*SP/2)
    sc = src // shard_real
    sv = src % shard_real
    HN = SP // 2
    shalf = sv // HN
    trow = shalf * (N_CORES * HN) + sc * HN + (sv - shalf * HN)
    bkt = trow % NBKT
    tq = trow // NBKT

    per_core = []
    dtot = []
    for c in range(N_CORES):
        rels = []
        dt_r = []
        for r in range(NUM_REL):
            bkts = []
            selr = (owner == c) & (et == r)
            dt_r.append(np.bincount(ldst[selr], minlength=SP))
            for b in range(NBKT):
                sel = selr & (bkt == b)
                l = ldst[sel]
                t = tq[sel]
                dcnt = np.bincount(l, minlength=SP)
                perm = np.argsort(-dcnt, kind="stable")
                rank = np.empty(SP, dtype=np.int64)
                rank[perm] = np.arange(SP)
                order = np.argsort(rank[l], kind="stable")
                l_s, t_s = l[order], t[order]
                s_sorted = rank[l_s]
                if len(l_s):
                    newgrp = np.r_[True, s_sorted[1:] != s_sorted[:-1]]
                    gidx = np.cumsum(newgrp) - 1
                    starts = np.flatnonzero(newgrp)
                    kpos = np.arange(len(l_s)) - starts[gidx]
                else:
                    kpos = np.zeros(0, dtype=np.int64)
                maxd = int(dcnt.max()) if len(l) else 0
                Lk = np.array([(dcnt > k).sum() for k in range(maxd)], dtype=np.int64)
                bkts.append(dict(rank=rank, s=s_sorted, k=kpos, t=t_s,
                                 maxd=maxd, Lk=Lk))
            rels.append(bkts)
        per_core.append(rels)
        dtot.append(dt_r)

    # global plane widths + piece decomposition per (r, b)
    pieces = [[None] * NBKT for _ in range(NUM_REL)]
    copy_cols = [[0] * NBKT for _ in range(NUM_REL)]
    colbase = [[None] * NBKT for _ in range(NUM_REL)]
    ixbase = [[None] * NBKT for _ in range(NUM_REL)]
    ubase = [[0] * NBKT for _ in range(NUM_REL)]
    g16 = 0
    for r in range(NUM_REL):
        for b in range(NBKT):
            maxd_g = max(per_core[c][r][b]["maxd"] for c in range(N_CORES))
            cks = []
            for k in range(maxd_g):
                m = 1
                for c in range(N_CORES):
                    Lk = per_core[c][r][b]["Lk"]
                    if k < len(Lk):
                        m = max(m, int(np.ceil(Lk[k] / 128)))
                cks.append(m)
            cb = []
            plist = []
            cur_ck = 0
            cur_adds = []
            col = 0
            bases = []
            for k, ck in enumerate(cks):
                cb.append(col)
                off = 0
                while off < ck:
                    room = CBUF - cur_ck
                    if room == 0:
                        plist.append((cur_ck, cur_adds))
                        cur_ck, cur_adds = 0, []
                        room = CBUF
                    take = min(room, ck - off)
                    cur_adds.append((off, cur_ck, take, k == 0))
                    cur_ck += take
                    off += take
                col += ck
            if cur_ck:
                plist.append((cur_ck, cur_adds))
            pieces[r][b] = plist
            colbase[r][b] = np.array(cb, dtype=np.int64)
            copy_cols[r][b] = cks[0] if cks else 0
            # idx16 col bases (8 cols of int16 per idx column)
            bases = []
            for (ck, _) in plist:
                bases.append(g16)
                g16 += ck * 8
            ixbase[r][b] = bases
    for r in range(NUM_REL):
        for b in range(NBKT):
            ubase[r][b] = g16
            g16 += SP // 16

    cfg = Cfg(shard_real, SP, pieces, copy_cols, ixbase, ubase, g16)
    return cfg, per_core, dtot, colbase


def _wrap16(seq):
    """int16 position stream -> [128, len/16] wrapped (p holds stream p%16::16)."""
    w = seq.reshape(-1, 16).T  # [16, cols]
    return w[np.arange(128) % 16, :]


def _prep(inputs, shard_real=12500, shard_pad=12800):
    SP = shard_pad
    cfg, per_core, dtot, colbase = _build_structures(
        inputs["edge_index"], inputs["edge_type"], shard_real, shard_pad)
    NT = cfg.nt

    f32 = np.float32
    bf16 = ml_dtypes.bfloat16

    wall = np.zeros((DTR, 32), dtype=bf16)
    wall[0:768, 0:8] = inputs["Wd"].astype(bf16)
    wall[768:1536, 8:16] = inputs["Wt"].astype(bf16)
    wall[1536:1542, 16:24] = inputs["Wn"].astype(bf16)
    wall[1542:1545, 24:32] = inputs["Wc"].astype(bf16)
    bx = np.zeros(32, dtype=f32)
    bx[0:8] = inputs["bd"]; bx[8:16] = inputs["bt"]
    bx[16:24] = inputs["bn"]; bx[24:32] = inputs["bc"]
    wall[1545, :] = bx.astype(bf16)

    wsm = np.zeros((6, 33, 32), dtype=bf16)
    wsm[0, 0:32] = inputs["Wi"].astype(bf16)
    wsm[0, 32] = inputs["bi"].astype(bf16)
    wsm[1, 0:32] = inputs["Wroot"].astype(bf16)
    wsm[1, 32] = inputs["brgcn"].astype(bf16)
    wsm[2, 0:32] = inputs["Wrel"][0].astype(bf16)
    wsm[3, 0:32] = inputs["Wrel"][1].astype(bf16)
    wsm[4, 0:32] = inputs["Wo1"].astype(bf16)
    wsm[4, 32] = inputs["bo1"].astype(bf16)
    wsm[5, 0:32, 0:2] = inputs["Wo2"].astype(bf16)
    wsm[5, 32, 0:2] = inputs["bo2"].astype(bf16)
    id128 = np.eye(128, dtype=np.float16)
    id32 = np.eye(32, dtype=bf16)

    in_maps = []
    for c in range(N_CORES):
        r0, r1 = c * shard_real, (c + 1) * shard_real
        dtb = np.zeros((DTR, SP), dtype=bf16)
        dtb[0:768, 0:shard_real] = inputs["des"][r0:r1].T.astype(bf16)
        dtb[768:1536, 0:shard_real] = inputs["tweet"][r0:r1].T.astype(bf16)
        dtb[1536:1542, 0:shard_real] = inputs["num_prop"][r0:r1].T.astype(bf16)
        dtb[1542:1545, 0:shard_real] = inputs["cat_prop"][r0:r1].T.astype(bf16)
        dtb[1545, 0:shard_real] = 1.0

        idx16 = np.zeros((128, cfg.t16), dtype=np.int16)
        for r in range(NUM_REL):
            for b in range(NBKT):
                d = per_core[c][r][b]
                plist = cfg.pieces[r][b]
                ncols = sum(ck for (ck, _) in plist)
                seq = np.full(ncols * 128, DUMMY_TQ, dtype=np.int16)
                if len(d["s"]):
                    cols = colbase[r][b][d["k"]] + d["s"] // 128
                    pos = cols * 128 + (d["s"] % 128)
                    seq[pos] = d["t"].astype(np.int16)
                off = 0
                for pi, (ck, _) in enumerate(plist):
                    blk = _wrap16(seq[off * 128:(off + ck) * 128])
                    base = cfg.ixbase[r][b][pi]
                    idx16[:, base:base + ck * 8] = blk
                    off += ck
                useq = d["rank"][np.arange(SP)].astype(np.int16)
                ub = cfg.ubase[r][b]
                idx16[:, ub:ub + SP // 16] = _wrap16(useq)

        coefnm = np.zeros((128, NUM_REL, NT), dtype=np.float16)
        for r in range(NUM_REL):
            cv = (1.0 / np.maximum(dtot[c][r], 1)).astype(np.float16)  # canonical order
            coefnm[:, r, :] = cv.reshape(NT, 128).T
        in_maps.append({
            "dtb": dtb, "wall": wall, "idx16": idx16, "coefnm": coefnm,
            "wsm": wsm, "id128": id128, "id32": id32,
        })
    return cfg, in_maps


_CACHE = {}
_RUN_KW = {}
_LAST_RES = None
_FIRST_RES = None


def kernel(**inputs):
    global _LAST_RES, _FIRST_RES
    cfg, in_maps = _prep(inputs)
    key = tuple((r, b, ck) for r in range(NUM_REL) for b in range(NBKT)
                for (ck, _) in cfg.pieces[r][b])
    if key not in _CACHE:
        _CACHE[key] = build_bass(cfg)
    nc = _CACHE[key]
    res = run_bass_kernel_spmd(nc, in_maps, list(range(N_CORES)), **_RUN_KW)
    _LAST_RES = res
    if _FIRST_RES is None:
        _FIRST_RES = res
    outs = []
    for c in range(N_CORES):
        o = res.results[c]["out"]
        outs.append(o.T[0:cfg.shard_real])
    return np.ascontiguousarray(np.concatenate(outs, axis=0).astype(np.float32))

